# revision 29
# baseline (speedup 1.0000x reference)
"""GATv2 layer kernel for 8 Trainium2 NeuronCores.

Math (reference is a GATv2 layer with N=8192 nodes, 128 in / 64 out feats):
    Wh  = mole_out @ W                      [N, 64]
    lr  = leakyrelu(Wh, 0.2)
    s1  = lr @ b[:64];  s2 = lr @ b[64:]
    e   = s1[:, None] + s2[None, :]         (masked by adj, row softmax)
    out = elu(softmax(e) @ Wh)

Key identity: s1[r] is constant along a softmax row, so it cancels:
    att[r, j] = adj[r, j] * exp(s2[j]) / sum_j adj[r, j] * exp(s2[j])
Let ev = exp(s2), G = diag(ev) @ Wh, H2 = [G | ev]  ([N, 65]).
Then raw[r, :] = sum_j adj[r, j] * H2[j, :]  and
    out[r, f] = elu(raw[r, f] / raw[r, 64]).
The whole attention collapses into one masked matmul against adj.

Sharding: rows (destination nodes) across 8 cores, 1024 rows each.  Each
core receives its adj slice TRANSPOSED and contiguous ([8192(j), 1024(r)]
int32) so the contraction index j lands on SBUF partitions; W / b / mole_out
are replicated.  Per-core device work:
  - pre-pass: Wh (+ a folded 0.2*(W@b2) column), relu, s2, ev, H2 (fp16)
  - main: stream adjT in [128, jpd*1024] i32 tiles (2MB HWDGE loads),
      cast to fp16 (DVE/ACT alternating; 0/1 is exact), accumulate
      psum[sb] [65, 512] += H2[jc].T @ adjT_tile[:, ...]
  - epilogue: divide by the ev-sum row, elu, store out^T [64, 1024].
Output is assembled on host by stacking the 8 row blocks.

Measured on 8 axon-tunneled TRN2 cores: main pass ~61 us/core
(~520 GB/s/core effective HBM read), estimated one-shot ~84 us.
"""

import numpy as np

import concourse.bacc as bacc
import concourse.mybir as mybir
import concourse.tile as tile
from concourse.bass_utils import run_bass_kernel_spmd

N = 8192          # nodes
C = 128           # input features
F = 64            # output features
NCORES = 8
RPC = N // NCORES  # rows (destination nodes) per core: 1024
ALPHA = 0.2

f32 = mybir.dt.float32
bf16 = mybir.dt.bfloat16
i32 = mybir.dt.int32
AF = mybir.ActivationFunctionType
ALU = mybir.AluOpType


def _emit(tc, n, rpc, cast_mode="engines", repeat=1, abf_bufs=4, jpd=2,
          a32_bufs=6, no_pre=False, mdt=bf16):
    """Emit the per-core program. n = total nodes, rpc = rows per core.

    repeat > 1 re-streams the whole adj pass that many times (psum restarts
    each pass, so outputs are unchanged) — used only to measure the
    steady-state main-loop time as a slope over repeat.
    """
    nc = tc.nc
    jt = n // 128          # number of j-chunks
    G = 4                  # Wh chunks per pre-pass group
    ng = jt // G
    nsb = rpc // 512       # superblocks of 512 destination rows
    F1 = F + 1

    adjT = nc.dram_tensor("adjT", [n, rpc], i32, kind="ExternalInput").ap()
    moleT = nc.dram_tensor("moleT", [C, n], f32, kind="ExternalInput").ap()
    Waug = nc.dram_tensor("Waug", [C, F1], f32, kind="ExternalInput").ap()
    b2r = nc.dram_tensor("b2r4", [128, G * F], f32, kind="ExternalInput").ap()
    outT = nc.dram_tensor("outT", [F, rpc], f32, kind="ExternalOutput").ap()

    with (
        tc.tile_pool(name="const", bufs=1) as const,
        tc.tile_pool(name="preps", bufs=4, space="PSUM") as pre_ps,
        tc.tile_pool(name="sml", bufs=3) as sml,
        tc.tile_pool(name="a32", bufs=a32_bufs) as a32p,
        tc.tile_pool(name="abf", bufs=abf_bufs) as abfp,
        tc.tile_pool(name="mainps", bufs=1, space="PSUM") as main_ps,
        tc.tile_pool(name="bcps", bufs=2, space="PSUM") as bc_ps,
        tc.tile_pool(name="epi", bufs=2) as epi,
    ):
        moleT_sb = const.tile([C, n], f32)
        nsplit = 4
        for s in range(nsplit):
            sl = slice(s * (n // nsplit), (s + 1) * (n // nsplit))
            nc.sync.dma_start(moleT_sb[:, sl], moleT[:, sl])
        W_sb = const.tile([C, F1], f32)
        nc.sync.dma_start(W_sb[:], Waug)
        b2_sb = const.tile([128, G * F], f32)
        nc.sync.dma_start(b2_sb[:], b2r)
        H2 = const.tile([128, jt * F1], mdt)
        ones_sb = const.tile([1, F], f32)
        nc.gpsimd.memset(ones_sb[:], 1.0)

        h2v = H2[:].rearrange("p (c f) -> p c f", f=F1)

        # ---- pre-pass: Wh, s2, ev, H2 = [ev*Wh | ev] ----
        if no_pre:  # timing-model experiment only: skip H2 construction
            nc.gpsimd.memset(H2[:], 0.0)
        for g in range(0 if no_pre else ng):
            ps = pre_ps.tile([128, G * F1], f32)
            for q in range(G):
                cc = g * G + q
                # [128(i), 65] = moleT[:, i-chunk].T @ [W | 0.2*W@b2]
                nc.tensor.matmul(
                    ps[:, q * F1:(q + 1) * F1],
                    lhsT=moleT_sb[:, cc * 128:(cc + 1) * 128],
                    rhs=W_sb[:],
                    start=True,
                    stop=True,
                )
            ps3 = ps[:].rearrange("p (g f) -> p g f", f=F1)
            wh4 = ps3[:, :, 0:F]          # [128, G, 64] Wh values
            sw4 = ps3[:, :, F:F1]         # [128, G, 1]  0.2*(Wh@b2)
            r4 = sml.tile([128, G * F], f32, tag="r4")
            nc.scalar.activation(r4[:], wh4, AF.Relu)
            m4 = sml.tile([128, G * F], f32, tag="m4")
            nc.vector.tensor_mul(m4[:], r4[:], b2_sb[:])   # b2_sb holds 0.8*b2
            sr4 = sml.tile([128, G], f32, tag="sr4")
            nc.vector.tensor_reduce(
                sr4[:], m4[:].rearrange("p (g f) -> p g f", f=F),
                axis=mybir.AxisListType.X, op=ALU.add,
            )
            s24 = sml.tile([128, G], f32, tag="s24")
            nc.vector.tensor_add(s24[:], sw4, sr4[:])      # s2 = 0.2*sw + 0.8*sr
            ev4 = sml.tile([128, G], f32, tag="ev4")
            nc.scalar.activation(ev4[:], s24[:], AF.Exp)
            for q in range(G):
                cc = g * G + q
                nc.scalar.mul(
                    h2v[:, cc:cc + 1, 0:F], ps3[:, q:q + 1, 0:F], ev4[:, q:q + 1]
                )
            nc.vector.tensor_copy(h2v[:, g * G:(g + 1) * G, F:F1], ev4[:])

        # ---- main: psum[sb] [65, 512] += H2[jc].T @ adjT_bf16 ----
        pss = [
            main_ps.tile([F1, 512], f32, name=f"mps{sb}", tag=f"mps{sb}")
            for sb in range(nsb)
        ]
        # adjT rows viewed as [jt, 128, rpc]; one DMA may carry `jpd` chunks.
        adjT3 = adjT.rearrange("(c p) r -> c p r", p=128)
        for rep in range(repeat):
            for jd in range(jt // jpd):
                src = adjT3[jd * jpd:(jd + 1) * jpd, :, :].rearrange(
                    "c p r -> p c r"
                )
                abf = abfp.tile([128, jpd * rpc], mdt)
                if cast_mode == "dma":
                    nc.gpsimd.dma_start(
                        abf[:].rearrange("p (c r) -> p c r", c=jpd), src
                    )
                else:
                    a32 = a32p.tile([128, jpd * rpc], i32)
                    nc.sync.dma_start(
                        a32[:].rearrange("p (c r) -> p c r", c=jpd), src
                    )
                    if jd % 2 == 0:
                        nc.vector.tensor_copy(abf[:], a32[:])
                    else:
                        nc.scalar.copy(abf[:], a32[:])
                for h in range(jpd):
                    jc = jd * jpd + h
                    for sb in range(nsb):
                        nc.tensor.matmul(
                            pss[sb][:],
                            lhsT=H2[:, jc * F1:(jc + 1) * F1],
                            rhs=abf[:, h * rpc + sb * 512:h * rpc + (sb + 1) * 512],
                            start=(jc == 0),
                            stop=(jc == jt - 1),
                        )

        # ---- epilogue: out = elu(num / den), stored transposed ----
        for sb in range(nsb):
            ps = pss[sb]
            rec = epi.tile([1, 512], f32, tag="rec")
            nc.vector.reciprocal(rec[:], ps[F:F1, :])
            bc = bc_ps.tile([F, 512], f32)
            nc.tensor.matmul(bc[:], lhsT=ones_sb[:], rhs=rec[:], start=True, stop=True)
            bc_sb = epi.tile([F, 512], f32, tag="bc")
            nc.vector.tensor_copy(bc_sb[:], bc[:])
            x = epi.tile([F, 512], f32, tag="x")
            nc.vector.tensor_mul(x[:], ps[0:F, :], bc_sb[:])
            mneg = epi.tile([F, 512], f32, tag="mneg")
            nc.vector.tensor_scalar_min(mneg[:], x[:], 0.0)
            e = epi.tile([F, 512], f32, tag="e")
            nc.scalar.activation(e[:], mneg[:], AF.Exp)
            r = epi.tile([F, 512], f32, tag="r")
            nc.scalar.activation(r[:], x[:], AF.Relu)
            o = epi.tile([F, 512], f32, tag="o")
            # o = (e + (-1)) + r  == elu(x)
            nc.vector.scalar_tensor_tensor(
                o[:], e[:], -1.0, r[:], op0=ALU.add, op1=ALU.add
            )
            nc.sync.dma_start(outT[:, sb * 512:(sb + 1) * 512], o[:])


_CACHE = {}


def _build(n=N, rpc=RPC, cast_mode="engines", repeat=1, abf_bufs=3, jpd=4,
           swdge_queues=1, a32_bufs=6, no_pre=False, mdt=mybir.dt.float16):
    key = (n, rpc, cast_mode, repeat, abf_bufs, jpd, swdge_queues, a32_bufs,
           no_pre, mdt)
    if key not in _CACHE:
        nc = bacc.Bacc(
            "TRN2", target_bir_lowering=False, debug=False, num_devices=NCORES,
            num_swdge_queues=swdge_queues,
        )
        with tile.TileContext(nc) as tc:
            _emit(tc, n, rpc, cast_mode, repeat, abf_bufs, jpd, a32_bufs,
                  no_pre, mdt)
        nc.compile()
        _CACHE[key] = nc
    return _CACHE[key]


def _host_prep(mole_out, adj, W, b, n=N, rpc=RPC, ncores=NCORES):
    mole_out = np.asarray(mole_out, dtype=np.float32)
    adj = np.asarray(adj, dtype=np.int32)
    W = np.asarray(W, dtype=np.float32)
    b = np.asarray(b, dtype=np.float32)
    b2 = b[F:]
    moleT = np.ascontiguousarray(mole_out.T)                     # [128, n]
    Waug = np.concatenate([W, (ALPHA * (W @ b2))[:, None]], axis=1)
    Waug = np.ascontiguousarray(Waug, dtype=np.float32)          # [128, 65]
    b2r4 = np.tile(((1.0 - ALPHA) * b2).astype(np.float32), (128, 4))
    b2r4 = np.ascontiguousarray(b2r4)                            # [128, 256]
    in_maps = []
    for k in range(ncores):
        adjTk = np.ascontiguousarray(adj[k * rpc:(k + 1) * rpc, :].T)
        in_maps.append(
            {"adjT": adjTk, "moleT": moleT, "Waug": Waug, "b2r4": b2r4}
        )
    return in_maps


def _run(inputs, trace=False, **kw):
    nc = _build()
    in_maps = _host_prep(**inputs)
    res = run_bass_kernel_spmd(
        nc, in_maps, core_ids=list(range(NCORES)), trace=trace, **kw
    )
    out = np.concatenate([r["outT"].T for r in res.results], axis=0)
    return np.ascontiguousarray(out, dtype=np.float32), res


def kernel(mole_out, adj, W, b):
    out, _ = _run(dict(mole_out=mole_out, adj=adj, W=W, b=b))
    return out


# revision 33
# speedup vs baseline: 1.1538x; 1.1538x over previous
"""GATv2 layer kernel for 8 Trainium2 NeuronCores.

Math (reference is a GATv2 layer with N=8192 nodes, 128 in / 64 out feats):
    Wh  = mole_out @ W                      [N, 64]
    lr  = leakyrelu(Wh, 0.2)
    s1  = lr @ b[:64];  s2 = lr @ b[64:]
    e   = s1[:, None] + s2[None, :]         (masked by adj, row softmax)
    out = elu(softmax(e) @ Wh)

Key identity: s1[r] is constant along a softmax row, so it cancels:
    att[r, j] = adj[r, j] * exp(s2[j]) / sum_j adj[r, j] * exp(s2[j])
Let ev = exp(s2), G = diag(ev) @ Wh, H2 = [G | ev]  ([N, 65]).
Then raw[r, :] = sum_j adj[r, j] * H2[j, :]  and
    out[r, f] = elu(raw[r, f] / raw[r, 64]).
The whole attention collapses into one masked matmul against adj.

Sharding: rows (destination nodes) across 8 cores, 1024 rows each.  Each
core receives its adj slice TRANSPOSED and contiguous ([8192(j), 1024(r)]
int32) so the contraction index j lands on SBUF partitions; W / b / mole_out
are replicated.  Per-core device work:
  - pre-pass: Wh (+ a folded 0.2*(W@b2) column), relu, s2, ev, H2 (fp16)
  - main: stream adjT in [128, jpd*1024] i32 tiles (2MB HWDGE loads),
      cast to fp16 (DVE/ACT alternating; 0/1 is exact), accumulate
      psum[sb] [65, 512] += H2[jc].T @ adjT_tile[:, ...]
  - epilogue: divide by the ev-sum row, elu, store out^T [64, 1024].
Output is assembled on host by stacking the 8 row blocks.

Measured on 8 axon-tunneled TRN2 cores: main pass ~61 us/core
(~520 GB/s/core effective HBM read), estimated one-shot ~84 us.
"""

import numpy as np

import concourse.bacc as bacc
import concourse.mybir as mybir
import concourse.tile as tile
from concourse.bass_utils import run_bass_kernel_spmd

N = 8192          # nodes
C = 128           # input features
F = 64            # output features
NCORES = 8
RPC = N // NCORES  # rows (destination nodes) per core: 1024
ALPHA = 0.2

f32 = mybir.dt.float32
bf16 = mybir.dt.bfloat16
i32 = mybir.dt.int32
AF = mybir.ActivationFunctionType
ALU = mybir.AluOpType


def _emit(tc, n, rpc, cast_mode="engines", repeat=1, abf_bufs=4, jpd=2,
          a32_bufs=6, no_pre=False, mdt=bf16, epi_bufs=2):
    """Emit the per-core program. n = total nodes, rpc = rows per core.

    repeat > 1 re-streams the whole adj pass that many times (psum restarts
    each pass, so outputs are unchanged) — used only to measure the
    steady-state main-loop time as a slope over repeat.
    """
    nc = tc.nc
    jt = n // 128          # number of j-chunks
    G = 4                  # Wh chunks per pre-pass group
    ng = jt // G
    nsb = rpc // 512       # superblocks of 512 destination rows
    F1 = F + 1

    adjT = nc.dram_tensor("adjT", [n, rpc], i32, kind="ExternalInput").ap()
    moleT = nc.dram_tensor("moleT", [C, n], f32, kind="ExternalInput").ap()
    Waug = nc.dram_tensor("Waug", [C, F1], f32, kind="ExternalInput").ap()
    b2r = nc.dram_tensor("b2r4", [128, G * F], f32, kind="ExternalInput").ap()
    outT = nc.dram_tensor("outT", [F, rpc], f32, kind="ExternalOutput").ap()

    with (
        tc.tile_pool(name="const", bufs=1) as const,
        tc.tile_pool(name="preps", bufs=4, space="PSUM") as pre_ps,
        tc.tile_pool(name="sml", bufs=3) as sml,
        tc.tile_pool(name="a32", bufs=a32_bufs) as a32p,
        tc.tile_pool(name="abf", bufs=abf_bufs) as abfp,
        tc.tile_pool(name="mainps", bufs=1, space="PSUM") as main_ps,
        tc.tile_pool(name="bcps", bufs=2, space="PSUM") as bc_ps,
        tc.tile_pool(name="epi", bufs=epi_bufs) as epi,
    ):
        moleT_sb = const.tile([C, n], f32)
        if no_pre:  # timing-model experiment only: skip the mole load too
            nc.gpsimd.memset(moleT_sb[:, 0:128], 0.0)
        else:
            nsplit = 4
            for s in range(nsplit):
                sl = slice(s * (n // nsplit), (s + 1) * (n // nsplit))
                nc.sync.dma_start(moleT_sb[:, sl], moleT[:, sl])
        W_sb = const.tile([C, F1], f32)
        nc.sync.dma_start(W_sb[:], Waug)
        b2_sb = const.tile([128, G * F], f32)
        nc.sync.dma_start(b2_sb[:], b2r)
        H2 = const.tile([128, jt * F1], mdt)
        ones_sb = const.tile([1, F], f32)
        nc.gpsimd.memset(ones_sb[:], 1.0)

        h2v = H2[:].rearrange("p (c f) -> p c f", f=F1)

        # ---- pre-pass: Wh, s2, ev, H2 = [ev*Wh | ev] ----
        if no_pre:  # timing-model experiment only: skip H2 construction
            nc.gpsimd.memset(H2[:], 0.0)
        for g in range(0 if no_pre else ng):
            ps = pre_ps.tile([128, G * F1], f32)
            for q in range(G):
                cc = g * G + q
                # [128(i), 65] = moleT[:, i-chunk].T @ [W | 0.2*W@b2]
                nc.tensor.matmul(
                    ps[:, q * F1:(q + 1) * F1],
                    lhsT=moleT_sb[:, cc * 128:(cc + 1) * 128],
                    rhs=W_sb[:],
                    start=True,
                    stop=True,
                )
            ps3 = ps[:].rearrange("p (g f) -> p g f", f=F1)
            wh4 = ps3[:, :, 0:F]          # [128, G, 64] Wh values
            sw4 = ps3[:, :, F:F1]         # [128, G, 1]  0.2*(Wh@b2)
            r4 = sml.tile([128, G * F], f32, tag="r4")
            nc.scalar.activation(r4[:], wh4, AF.Relu)
            m4 = sml.tile([128, G * F], f32, tag="m4")
            nc.vector.tensor_mul(m4[:], r4[:], b2_sb[:])   # b2_sb holds 0.8*b2
            sr4 = sml.tile([128, G], f32, tag="sr4")
            nc.vector.tensor_reduce(
                sr4[:], m4[:].rearrange("p (g f) -> p g f", f=F),
                axis=mybir.AxisListType.X, op=ALU.add,
            )
            s24 = sml.tile([128, G], f32, tag="s24")
            nc.vector.tensor_add(s24[:], sw4, sr4[:])      # s2 = 0.2*sw + 0.8*sr
            ev4 = sml.tile([128, G], f32, tag="ev4")
            nc.scalar.activation(ev4[:], s24[:], AF.Exp)
            for q in range(G):
                cc = g * G + q
                nc.scalar.mul(
                    h2v[:, cc:cc + 1, 0:F], ps3[:, q:q + 1, 0:F], ev4[:, q:q + 1]
                )
            nc.vector.tensor_copy(h2v[:, g * G:(g + 1) * G, F:F1], ev4[:])

        # ---- main: psum[sb] [65, 512] += H2[jc].T @ adjT_bf16 ----
        pss = [
            main_ps.tile([F1, 512], f32, name=f"mps{sb}", tag=f"mps{sb}")
            for sb in range(nsb)
        ]
        # adjT rows viewed as [jt, 128, rpc]; one DMA may carry `jpd` chunks.
        adjT3 = adjT.rearrange("(c p) r -> c p r", p=128)
        for rep in range(repeat):
            for jd in range(jt // jpd):
                src = adjT3[jd * jpd:(jd + 1) * jpd, :, :].rearrange(
                    "c p r -> p c r"
                )
                abf = abfp.tile([128, jpd * rpc], mdt)
                if cast_mode == "dma":
                    nc.gpsimd.dma_start(
                        abf[:].rearrange("p (c r) -> p c r", c=jpd), src
                    )
                else:
                    a32 = a32p.tile([128, jpd * rpc], i32)
                    nc.sync.dma_start(
                        a32[:].rearrange("p (c r) -> p c r", c=jpd), src
                    )
                    # split each cast across DVE and ACT: halves the
                    # pipeline-fill latency and balances both engines
                    half = jpd * rpc // 2
                    nc.vector.tensor_copy(abf[:, :half], a32[:, :half])
                    nc.scalar.copy(abf[:, half:], a32[:, half:])
                for h in range(jpd):
                    jc = jd * jpd + h
                    for sb in range(nsb):
                        nc.tensor.matmul(
                            pss[sb][:],
                            lhsT=H2[:, jc * F1:(jc + 1) * F1],
                            rhs=abf[:, h * rpc + sb * 512:h * rpc + (sb + 1) * 512],
                            start=(jc == 0),
                            stop=(jc == jt - 1),
                        )

        # ---- epilogue: out = elu(num / den), stored transposed ----
        for sb in range(nsb):
            ps = pss[sb]
            rec = epi.tile([1, 512], f32, tag="rec")
            nc.vector.reciprocal(rec[:], ps[F:F1, :])
            bc = bc_ps.tile([F, 512], f32)
            nc.tensor.matmul(bc[:], lhsT=ones_sb[:], rhs=rec[:], start=True, stop=True)
            bc_sb = epi.tile([F, 512], f32, tag="bc")
            nc.vector.tensor_copy(bc_sb[:], bc[:])
            x = epi.tile([F, 512], f32, tag="x")
            nc.vector.tensor_mul(x[:], ps[0:F, :], bc_sb[:])
            mneg = epi.tile([F, 512], f32, tag="mneg")
            nc.vector.tensor_scalar_min(mneg[:], x[:], 0.0)
            e = epi.tile([F, 512], f32, tag="e")
            nc.scalar.activation(e[:], mneg[:], AF.Exp)
            r = epi.tile([F, 512], f32, tag="r")
            nc.scalar.activation(r[:], x[:], AF.Relu)
            o = epi.tile([F, 512], f32, tag="o")
            # o = (e + (-1)) + r  == elu(x)
            nc.vector.scalar_tensor_tensor(
                o[:], e[:], -1.0, r[:], op0=ALU.add, op1=ALU.add
            )
            nc.sync.dma_start(outT[:, sb * 512:(sb + 1) * 512], o[:])


_CACHE = {}


def _build(n=N, rpc=RPC, cast_mode="engines", repeat=1, abf_bufs=3, jpd=4,
           swdge_queues=1, a32_bufs=6, no_pre=False, mdt=mybir.dt.float16,
           epi_bufs=2):
    key = (n, rpc, cast_mode, repeat, abf_bufs, jpd, swdge_queues, a32_bufs,
           no_pre, mdt, epi_bufs)
    if key not in _CACHE:
        nc = bacc.Bacc(
            "TRN2", target_bir_lowering=False, debug=False, num_devices=NCORES,
            num_swdge_queues=swdge_queues,
        )
        with tile.TileContext(nc) as tc:
            _emit(tc, n, rpc, cast_mode, repeat, abf_bufs, jpd, a32_bufs,
                  no_pre, mdt, epi_bufs)
        nc.compile()
        _CACHE[key] = nc
    return _CACHE[key]


def _host_prep(mole_out, adj, W, b, n=N, rpc=RPC, ncores=NCORES):
    mole_out = np.asarray(mole_out, dtype=np.float32)
    adj = np.asarray(adj, dtype=np.int32)
    W = np.asarray(W, dtype=np.float32)
    b = np.asarray(b, dtype=np.float32)
    b2 = b[F:]
    moleT = np.ascontiguousarray(mole_out.T)                     # [128, n]
    Waug = np.concatenate([W, (ALPHA * (W @ b2))[:, None]], axis=1)
    Waug = np.ascontiguousarray(Waug, dtype=np.float32)          # [128, 65]
    b2r4 = np.tile(((1.0 - ALPHA) * b2).astype(np.float32), (128, 4))
    b2r4 = np.ascontiguousarray(b2r4)                            # [128, 256]
    in_maps = []
    for k in range(ncores):
        adjTk = np.ascontiguousarray(adj[k * rpc:(k + 1) * rpc, :].T)
        in_maps.append(
            {"adjT": adjTk, "moleT": moleT, "Waug": Waug, "b2r4": b2r4}
        )
    return in_maps


def _run(inputs, trace=False, **kw):
    nc = _build()
    in_maps = _host_prep(**inputs)
    res = run_bass_kernel_spmd(
        nc, in_maps, core_ids=list(range(NCORES)), trace=trace, **kw
    )
    out = np.concatenate([r["outT"].T for r in res.results], axis=0)
    return np.ascontiguousarray(out, dtype=np.float32), res


def kernel(mole_out, adj, W, b):
    out, _ = _run(dict(mole_out=mole_out, adj=adj, W=W, b=b))
    return out
